# revision 16
# baseline (speedup 1.0000x reference)
"""DoubleAttention Trainium2 kernel — fp8 DoubleRow edition, data-parallel
over batch across 8 cores (2 samples/core).

Math per sample (C=512, KC=256, VC=512, H=8 heads, L=4096):
  K = Wk@X, Q = Wq@X, V = Wv@X          (1x1 convs as matmuls)
  key_sm = softmax_L(K)  (bk per-row shift: no-op)
  q_sm   = softmax_head32(Q + bq)
  ctx_h = V_h @ key_sm_h^T ; att = ctx @ q_sm ; out = x + We@att + wbias

fp8 strategy (rel-err budget 2e-2; attention term is ~1% of |out|):
  - All big matmuls in fp8e4m3 with DoubleRow perf mode (2 K-slices per
    instruction, 2x bf16 rate): K/V proj pair input-channel chunks,
    ctx/skt pair L-tiles, Q pairs channel chunks, output pairs mt halves.
  - Weights pre-scaled x16 on host so fp8 quantization stays in normal
    range; exp() applies scale=1/16; other x16 factors cancel or are
    folded into the final 1/2048 output descale.
  - exp shifted by -1.5 (softmax-invariant) so fp8 eq/ek never overflow.
  - Residual path at fp16: x loaded as fp16 (wbias pre-added on host),
    added either on DVE (scalar_tensor_tensor) or via a 2048*I fp16
    identity matmul into PSUM + ACT copy(scale=1/2048); output fp16.
Schedule: s1 stage-1 interleaved with s0 phase-A (same as f32r baseline).
"""

import numpy as np

_CACHE = {}

N_CORES = 8
N, C, Hdim, Wdim = 16, 512, 64, 64
L = Hdim * Wdim            # 4096
KC, VC = 256, 512
NH = 8                     # heads
HV = VC // NH              # 64 head value channels
S_PER_CORE = N // N_CORES  # 2 samples per core
NB = L // 512              # 8 L-banks of 512
NT = L // 128              # 32 L-tiles of 128
NP = NT // 2               # 16 L-tile pairs

WS = 16.0                  # host weight pre-scale
ESH = -1.5                 # exp shift (softmax-invariant)
QS = 32.0                  # qs = 32*qsm  (fp8 range)
MTS = 0.25                 # mt8 = 0.25 * (256*M) = 64*M
ODS = 1.0 / 2048.0         # output descale: 64*32


def _build_nc():
    import concourse.mybir as mybir
    import concourse.tile as tile
    from concourse import bacc

    F32 = mybir.dt.float32
    F16 = mybir.dt.float16
    F8 = mybir.dt.float8e4
    AF = mybir.ActivationFunctionType
    ALU = mybir.AluOpType
    DR = mybir.MatmulPerfMode.DoubleRow

    nc = bacc.Bacc("TRN2", target_bir_lowering=False, debug=False)

    xin8 = nc.dram_tensor("xin8", [S_PER_CORE * C, L], F8, kind="ExternalInput")
    xin16 = nc.dram_tensor("xin16", [S_PER_CORE * C, L], F16,
                           kind="ExternalInput")
    wk8_d = nc.dram_tensor("wk8", [128, 4, KC], F8, kind="ExternalInput")
    wq8_d = nc.dram_tensor("wq8", [128, 4, KC], F8, kind="ExternalInput")
    wv8_d = nc.dram_tensor("wv8", [128, 4, VC], F8, kind="ExternalInput")
    we8_d = nc.dram_tensor("we8", [128, 4, C], F8, kind="ExternalInput")
    bq_d = nc.dram_tensor("bqv", [128, 2], F32, kind="ExternalInput")
    esh_d = nc.dram_tensor("eshv", [128, 1], F32, kind="ExternalInput")
    bs_d = nc.dram_tensor("bsum8", [128, 128], F8, kind="ExternalInput")
    id_d = nc.dram_tensor("ident", [128, 128], F32, kind="ExternalInput")
    idr_d = nc.dram_tensor("idr16", [128, 128], F16, kind="ExternalInput")
    ones_d = nc.dram_tensor("ones8", [128, 2, 16], F8, kind="ExternalInput")
    xT8_d = nc.dram_tensor("xT8", [S_PER_CORE * L, C], F8, kind="ExternalInput")
    id16_d = nc.dram_tensor("ident16", [128, 128], F16, kind="ExternalInput")
    onr_d = nc.dram_tensor("onesr", [1, 128], F32, kind="ExternalInput")
    out_d = nc.dram_tensor("out", [S_PER_CORE * C, L], F16,
                           kind="ExternalOutput")

    with tile.TileContext(nc) as tc:
        with tc.tile_pool(name="wpool", bufs=1) as wp, \
             tc.tile_pool(name="work", bufs=1) as sp, \
             tc.tile_pool(name="ppool", bufs=1, space="PSUM") as pp:

            # ---- resident weights/constants ----
            wk8 = wp.tile([128, 4, KC], F8, name="wk8_s")
            wq8 = wp.tile([128, 4, KC], F8, name="wq8_s")
            wv8 = wp.tile([128, 4, VC], F8, name="wv8_s")
            we8 = wp.tile([128, 4, C], F8, name="we8_s")
            bsum8 = wp.tile([128, 128], F8, name="bs_s")
            ident = wp.tile([128, 128], F32, name="id_s")
            idr16 = wp.tile([128, 128], F16, name="idr_s")
            ones8 = wp.tile([128, 2, 16], F8, name="ones_s")
            bq2 = wp.tile([128, 2], F32, name="bq_s")
            esh = wp.tile([128, 1], F32, name="esh_s")
            ident16 = wp.tile([128, 128], F16, name="id16_s")
            onesr = wp.tile([1, 128], F32, name="onr_s")

            # first x tile goes out before the weights so DMA queues overlap
            x4_first = sp.tile([128, 4, 512], F8, name="x4_0_0", tag="x4",
                               bufs=4)
            for c in range(4):
                nc.gpsimd.dma_start(
                    out=x4_first[:, c, :],
                    in_=xin8[c * 128:(c + 1) * 128, 0:512])
                nc.sync.dma_start(out=wk8[:, c, :], in_=wk8_d[:, c, :])
                nc.sync.dma_start(out=wv8[:, c, :], in_=wv8_d[:, c, :])
            for dst, src in ((wq8, wq8_d), (we8, we8_d), (bsum8, bs_d),
                             (ident, id_d), (idr16, idr_d), (ones8, ones_d),
                             (bq2, bq_d), (esh, esh_d), (ident16, id16_d),
                             (onesr, onr_d)):
                nc.sync.dma_start(out=dst, in_=src[...])

            # dummy exp to force the one-time ACT_TABLE_LOAD into the
            # DMA lead-in window; memset source so it depends on no DMA
            # (a DMA-dependent warm op inherits the batched DMA semaphore
            # and stalls the whole in-order ACT queue behind the prefetches)
            warm = sp.tile([128, 2], F32, name="actwarm", tag="warm", bufs=1)
            nc.vector.memset(warm[:, :], 0.0)
            nc.scalar.activation(warm[:, 1:2], warm[:, 0:1], AF.Exp,
                                 bias=warm[:, 0:1])

            st = {}   # per-sample state: ctx_ps, skt_ps, mt8

            def load_xT(s, pr):
                xT2 = sp.tile([128, 2, C], F8, name=f"xt{s}_{pr}",
                              tag="xts", bufs=4)
                nc.gpsimd.dma_start(
                    out=xT2,
                    in_=xT8_d[s * L + pr * 256:s * L + (pr + 1) * 256, :]
                    .rearrange("(two p) c -> p two c", p=128))
                st[s][("xt", pr)] = xT2

            bt_pend = []

            def stage1_pair(s, x4, pr, ctx_ps, skt_ps):
                # kt+exp per tile; DoubleRow B^T/skt accumulation is deferred
                # one pair (software pipeline) so exp latency hides under the
                # next pair's kt matmuls
                ekT2 = sp.tile([128, 2, KC], F8, name=f"ek{s}_{pr}",
                               tag="ek", bufs=4)
                xT2 = st[s][("xt", pr)]
                for i in (0, 1):
                    k = (2 * pr + i) % 4          # tile within group
                    ksl = slice(k * 128, (k + 1) * 128)
                    kt_ps = pp.tile([128, KC], F32, name=f"kt{s}_{pr}_{i}",
                                    tag="kt", bufs=2)
                    for cp in (0, 1):
                        cs = slice(2 * cp, 2 * cp + 2)
                        nc.tensor.matmul(kt_ps[:, :], x4[:, cs, ksl],
                                         wk8[:, cs, :], start=(cp == 0),
                                         stop=(cp == 1), perf_mode=DR)
                    nc.scalar.activation(ekT2[:, i, :], kt_ps[:, :], AF.Exp,
                                         bias=esh[:, 0:1], scale=1.0 / WS)
                def bt_accum():
                    for j in (0, 1):
                        jsl = slice(j * 128, (j + 1) * 128)
                        nc.tensor.matmul(ctx_ps[j][:, :], ekT2[:, :, jsl],
                                         xT2[:, :, :],
                                         start=(pr == 0), stop=(pr == NP - 1),
                                         perf_mode=DR)
                    nc.tensor.matmul(skt_ps[:, :], ones8[:, :, 0:1],
                                     ekT2[:, :, :],
                                     start=(pr == 0), stop=(pr == NP - 1),
                                     perf_mode=DR)
                bt_pend.append(bt_accum)
                if len(bt_pend) > 1:
                    bt_pend.pop(0)()
                if pr == NP - 1:
                    while bt_pend:
                        bt_pend.pop(0)()

            def stage1_group(s, g, pairs=(0, 1)):
                row0 = s * C
                if g == 0 and pairs[0] == 0:
                    st[s] = dict(
                        ctx_ps=[pp.tile([128, C], F32, name=f"ctx{s}_{j}",
                                        tag="ctx", bufs=2) for j in range(2)],
                        skt_ps=pp.tile([1, KC], F32, name=f"skt{s}",
                                       tag="skt", bufs=2))
                def load_x4(gg):
                    if s == 0 and gg == 0:
                        st[s][("x4", gg)] = x4_first
                        return
                    t = sp.tile([128, 4, 512], F8, name=f"x4_{s}_{gg}",
                                tag="x4", bufs=4)
                    nc.gpsimd.dma_start(
                        out=t,
                        in_=xin8[row0:row0 + C, gg * 512:(gg + 1) * 512]
                        .rearrange("(c p) l -> p c l", p=128))
                    st[s][("x4", gg)] = t
                if pairs[0] == 0:
                    if g == 0:
                        load_x4(0)
                        load_xT(s, 0)
                        load_xT(s, 1)
                    if g + 1 < NB:
                        load_x4(g + 1)
                        load_xT(s, 2 * g + 2)
                        load_xT(s, 2 * g + 3)
                x4 = st[s][("x4", g)]
                ctx_ps, skt_ps = st[s]["ctx_ps"], st[s]["skt_ps"]
                for p_ in pairs:
                    stage1_pair(s, x4, 2 * g + p_, ctx_ps, skt_ps)

            def mid(s):
                # B-route: bt_ps[j] holds B^T KC-chunk j = [128 k, 512 c].
                # 1) rk row; 2) B^T -> fp8 -> transpose -> B [c-part, k];
                # 3) ctx = (16Wv) @ B per v-chunk; 4) mask+normalize -> cn;
                # 5) mt = cn-pairs @ we8 (DoubleRow), as before.
                bt_ps, skt_ps = st[s]["ctx_ps"], st[s]["skt_ps"]
                sk_sb = sp.tile([1, KC], F32, name=f"sksb{s}", tag="sksb",
                                bufs=2)
                nc.vector.tensor_copy(sk_sb[:, :], skt_ps[:, :])
                rkrow = sp.tile([1, KC], F32, name=f"rkr{s}", tag="rk",
                                bufs=2)
                nc.vector.reciprocal(rkrow[:, :], sk_sb[:, :])
                rkb_ps = pp.tile([128, KC], F32, name=f"rkb{s}", tag="skt",
                                 bufs=2)
                nc.tensor.matmul(rkb_ps[:, :], onesr[0:1, :], rkrow[0:1, :],
                                 start=True, stop=True)
                rkb = sp.tile([128, KC], F32, name=f"rkbs{s}", tag="rkb",
                              bufs=2)
                nc.vector.tensor_copy(rkb[:, :], rkb_ps[:, :])
                btX = sp.tile([128, 2, C], F16, name=f"btx{s}", tag="btx",
                              bufs=2)
                for j in range(2):
                    nc.scalar.activation(btX[:, j, :], bt_ps[j][:, :],
                                         AF.Copy, scale=0.125)
                # transpose B^T -> B: 8 fp8 128x128 transposes
                bX = sp.tile([128, 4, KC], F8, name=f"bx{s}", tag="bx",
                             bufs=2)
                for cc in range(4):
                    csl = slice(cc * 128, (cc + 1) * 128)
                    tr8 = pp.tile([128, 2, 128], F16, name=f"tr8{s}_{cc}",
                                  tag="kt", bufs=2)
                    for j in range(2):
                        nc.tensor.transpose(tr8[:, j, :], btX[:, j, csl],
                                            ident16[:, :])
                    nc.vector.tensor_copy(bX[:, cc, :],
                                          tr8[:, :, :].rearrange(
                                              "p two q -> p (two q)"))
                # ctx[v, k] per v-chunk vc: contract over c (DR pairs)
                cn = sp.tile([128, 4, KC], F8, name=f"cn{s}", tag="cn",
                             bufs=2)
                nc.vector.memset(cn[:, :, :], 0.0)
                for vc in range(4):
                    vsl = slice(vc * 128, (vc + 1) * 128)
                    cx_ps = pp.tile([128, KC], F32, name=f"cx{s}_{vc}",
                                    tag="vt", bufs=2)
                    for ccp in (0, 1):
                        cs = slice(2 * ccp, 2 * ccp + 2)
                        nc.tensor.matmul(cx_ps[:, :], wv8[:, cs, vsl],
                                         bX[:, cs, :], start=(ccp == 0),
                                         stop=(ccp == 1), perf_mode=DR)
                    # normalize+mask the two head blocks living in chunk vc
                    for hh in range(2):
                        h = 2 * vc + hh
                        pr_ = slice(hh * 64, hh * 64 + 64)
                        kr = slice(h * 32, h * 32 + 32)
                        nc.vector.tensor_mul(cn[pr_, vc, kr],
                                             cx_ps[pr_, kr], rkb[pr_, kr])
                mt8 = sp.tile([128, 2, C], F8, name=f"mt{s}", tag="mt",
                              bufs=2)
                for j in range(2):
                    jsl = slice(j * 128, (j + 1) * 128)
                    mt_ps = pp.tile([128, C], F32, name=f"mtp{s}_{j}",
                                    tag="vt", bufs=2)
                    nc.tensor.matmul(mt_ps[:, :], cn[:, 2 * j:2 * j + 2, jsl],
                                     we8[:, 2 * j:2 * j + 2, :],
                                     start=True, stop=True, perf_mode=DR)
                    nc.scalar.activation(mt8[:, j, :], mt_ps[:, :], AF.Copy,
                                         scale=MTS)
                st[s]["mt8"] = mt8

            pend = []

            def _softmaxA(s, b):
                row0 = s * C
                bsl = slice(b * 512, (b + 1) * 512)
                xb8 = sp.tile([128, 4, 512], F8, name=f"xb8{s}_{b}",
                              tag="xb8", bufs=4)
                nc.gpsimd.dma_start(
                    out=xb8,
                    in_=xin8[row0:row0 + C, bsl]
                    .rearrange("(c p) l -> p c l", p=128))
                xb16 = sp.tile([128, 4, 512], F16, name=f"xb{s}_{b}",
                               tag="xb", bufs=4)
                nc.gpsimd.dma_start(
                    out=xb16,
                    in_=xin16[row0:row0 + C, bsl]
                    .rearrange("(c p) l -> p c l", p=128))
                qs2 = sp.tile([128, 2, 512], F8, name=f"qs{s}_{b}",
                              tag="qs", bufs=6)
                eqs = []
                for j in range(2):
                    jsl = slice(j * 128, (j + 1) * 128)
                    q_ps = pp.tile([128, 512], F32, name=f"q{s}_{b}_{j}",
                                   tag="kt", bufs=2)
                    for cp in (0, 1):
                        cs = slice(2 * cp, 2 * cp + 2)
                        nc.tensor.matmul(q_ps[:, :], wq8[:, cs, jsl],
                                         xb8[:, cs, :], start=(cp == 0),
                                         stop=(cp == 1), perf_mode=DR)
                    eq = sp.tile([128, 512], F8, name=f"eq{s}_{b}_{j}",
                                 tag="eq", bufs=6)
                    nc.scalar.activation(eq[:, :], q_ps[:, :], AF.Exp,
                                         bias=bq2[:, j:j + 1],
                                         scale=1.0 / WS)
                    eqs.append(eq)
                for j in range(2):
                    sq_ps = pp.tile([128, 512], F32, name=f"sq{s}_{b}_{j}",
                                    tag="skt", bufs=2)
                    nc.tensor.matmul(sq_ps[:, :], bsum8[:, :], eqs[j][:, :],
                                     start=True, stop=True)
                    rf = sp.tile([128, 512], F32, name=f"rf{s}_{b}_{j}",
                                 tag="rf", bufs=4)
                    nc.vector.reciprocal_approx_fast(rf[:, :], sq_ps[:, :])
                    nc.vector.scalar_tensor_tensor(
                        out=qs2[:, j, :], in0=eqs[j][:, :], scalar=QS,
                        in1=rf[:, :], op0=ALU.mult, op1=ALU.mult)
                return xb16, qs2

            def _outputA(s, b, xb16, qs2):
                row0 = s * C
                mt8 = st[s]["mt8"]
                bsl = slice(b * 512, (b + 1) * 512)
                for c in range(4):
                    o_ps = pp.tile([128, 512], F32, name=f"o{s}_{b}_{c}",
                                   tag="vt", bufs=2)
                    csl = slice(c * 128, (c + 1) * 128)
                    if c < 1:
                        # residual on DVE
                        nc.tensor.matmul(o_ps[:, :], mt8[:, :, csl],
                                         qs2[:, :, :], start=True, stop=True,
                                         perf_mode=DR)
                        oc = sp.tile([128, 512], F16, name=f"oc{s}_{b}_{c}",
                                     tag="oc", bufs=6)
                        nc.vector.scalar_tensor_tensor(
                            out=oc[:, :], in0=o_ps[:, :], scalar=ODS,
                            in1=xb16[:, c, :], op0=ALU.mult, op1=ALU.add)
                    else:
                        # residual folded into PSUM via 2048*I fp16 matmul,
                        # descale via ACT copy
                        nc.tensor.matmul(o_ps[:, :], mt8[:, :, csl],
                                         qs2[:, :, :], start=True, stop=False,
                                         perf_mode=DR)
                        nc.tensor.matmul(o_ps[:, :], idr16[:, :],
                                         xb16[:, c, :],
                                         start=False, stop=True)
                        oc = sp.tile([128, 512], F16, name=f"oc{s}_{b}_{c}",
                                     tag="oc", bufs=6)
                        nc.scalar.activation(oc[:, :], o_ps[:, :], AF.Copy,
                                             scale=ODS)
                    nc.sync.dma_start(
                        out=out_d[row0 + c * 128:row0 + (c + 1) * 128, bsl],
                        in_=oc[:, :])

            def phaseA_bank(s, b):
                pend.append((s, b) + _softmaxA(s, b))
                if len(pend) > 2:
                    _outputA(*pend.pop(0))

            def phaseA_flush():
                while pend:
                    _outputA(*pend.pop(0))

            # schedule: s0 stage1; mid(0); s1 stage1 interleaved with six
            # s0 phase-A banks (PE-rich stage1 complements engine-balanced
            # phase-A); mid(1); tail = s1 banks + s0 banks 6,7
            for g in range(NB):
                stage1_group(0, g)
            stage1_group(1, 0)
            mid(0)          # s1 group 0 gives the scheduler PE filler here
            stage1_group(1, 1)
            for i in range(2, NB):
                stage1_group(1, i, pairs=(0,))
                pend.append((0, i - 2) + _softmaxA(0, i - 2))
                stage1_group(1, i, pairs=(1,))
                if len(pend) > 2:
                    _outputA(*pend.pop(0))
            pend.append((0, 6) + _softmaxA(0, 6))
            mid(1)          # bank (0,6) softmax fills mid(1)'s chain
            for i in range(NB):
                phaseA_bank(1, i)
                if i == 1:
                    phaseA_bank(0, 7)
            phaseA_flush()
    nc.compile()
    return nc


def _host_prep(Wk, bk, Wq, bq, Wv, bv, We, be):
    import ml_dtypes
    f = np.float32
    F8 = ml_dtypes.float8_e4m3

    def chunk8(w):                  # (O, Cin) -> (128, Cin//128, O) fp8, x16
        wt = np.ascontiguousarray(w.T.astype(np.float64) * WS)
        wt = np.clip(wt, -240.0, 240.0)
        nch = wt.shape[0] // 128
        return np.ascontiguousarray(
            wt.reshape(nch, 128, w.shape[0]).transpose(1, 0, 2)).astype(F8)

    wk8 = chunk8(Wk)
    wq8 = chunk8(Wq)
    wv8 = chunk8(Wv)
    we8 = chunk8(We)
    bq2 = np.ascontiguousarray(
        bq.astype(f).reshape(2, 128).T) + np.float32(ESH)
    wb = (We.astype(np.float64) @ bv.astype(np.float64)
          + be.astype(np.float64))
    bsum = np.zeros((128, 128), f)
    for p in range(128):
        bsum[p, (p // 32) * 32:(p // 32) * 32 + 32] = 1.0
    ident = np.eye(128, dtype=f)
    idr16 = (np.eye(128) * 2048.0).astype(np.float16)
    ones8 = np.ones((128, 2, 16), dtype=F8)
    eshv = np.full((128, 1), ESH, dtype=f)
    onesr = np.full((1, 128), 8.0, dtype=f)
    return dict(wk8=wk8, wq8=wq8, wv8=wv8, we8=we8, bqv=bq2, eshv=eshv,
                bsum8=bsum.astype(F8), ident=ident, ident16=ident.astype(np.float16),
                idr16=idr16, ones8=ones8, onesr=onesr), wb


def _make_in_maps(x, Wk, bk, Wq, bq, Wv, bv, We, be):
    import ml_dtypes
    F8 = ml_dtypes.float8_e4m3
    shared, wb = _host_prep(Wk, bk, Wq, bq, Wv, bv, We, be)
    xf = np.ascontiguousarray(x.astype(np.float64).reshape(N, C, L))
    x8 = np.clip(xf, -240.0, 240.0).astype(F8)
    x16 = (xf + wb[None, :, None]).astype(np.float16)
    in_maps = []
    for i in range(N_CORES):
        m = dict(shared)
        sl = slice(i * S_PER_CORE, (i + 1) * S_PER_CORE)
        m["xin8"] = np.ascontiguousarray(
            x8[sl].reshape(S_PER_CORE * C, L))
        m["xin16"] = np.ascontiguousarray(
            x16[sl].reshape(S_PER_CORE * C, L))
        m["xT8"] = np.ascontiguousarray(
            x8[sl].transpose(0, 2, 1).reshape(S_PER_CORE * L, C))
        in_maps.append(m)
    return in_maps


def kernel(x, Wk, bk, Wq, bq, Wv, bv, We, be):
    from concourse.bass_utils import run_bass_kernel_spmd

    assert x.shape == (N, C, Hdim, Wdim), x.shape
    if "nc" not in _CACHE:
        _CACHE["nc"] = _build_nc()
    nc = _CACHE["nc"]

    in_maps = _make_in_maps(x, Wk, bk, Wq, bq, Wv, bv, We, be)
    res = run_bass_kernel_spmd(nc, in_maps, core_ids=list(range(N_CORES)))
    out = np.concatenate(
        [np.asarray(res.results[i]["out"], dtype=np.float32)
         .reshape(S_PER_CORE, C, Hdim, Wdim)
         for i in range(N_CORES)], axis=0)
    return out.astype(np.float32)


# revision 17
# speedup vs baseline: 1.1597x; 1.1597x over previous
"""DoubleAttention Trainium2 kernel — fp8 DoubleRow edition, data-parallel
over batch across 8 cores (2 samples/core).

Math per sample (C=512, KC=256, VC=512, H=8 heads, L=4096):
  K = Wk@X, Q = Wq@X, V = Wv@X          (1x1 convs as matmuls)
  key_sm = softmax_L(K)  (bk per-row shift: no-op)
  q_sm   = softmax_head32(Q + bq)
  ctx_h = V_h @ key_sm_h^T ; att = ctx @ q_sm ; out = x + We@att + wbias

fp8 strategy (rel-err budget 2e-2; attention term is ~1% of |out|):
  - All big matmuls in fp8e4m3 with DoubleRow perf mode (2 K-slices per
    instruction, 2x bf16 rate): K/V proj pair input-channel chunks,
    ctx/skt pair L-tiles, Q pairs channel chunks, output pairs mt halves.
  - Weights pre-scaled x16 on host so fp8 quantization stays in normal
    range; exp() applies scale=1/16; other x16 factors cancel or are
    folded into the final 1/2048 output descale.
  - exp shifted by -1.5 (softmax-invariant) so fp8 eq/ek never overflow.
  - Residual path at fp16: x loaded as fp16 (wbias pre-added on host),
    added either on DVE (scalar_tensor_tensor) or via a 2048*I fp16
    identity matmul into PSUM + ACT copy(scale=1/2048); output fp16.
Schedule: s1 stage-1 interleaved with s0 phase-A (same as f32r baseline).
"""

import numpy as np

_CACHE = {}

N_CORES = 8
N, C, Hdim, Wdim = 16, 512, 64, 64
L = Hdim * Wdim            # 4096
KC, VC = 256, 512
NH = 8                     # heads
HV = VC // NH              # 64 head value channels
S_PER_CORE = N // N_CORES  # 2 samples per core
NB = L // 512              # 8 L-banks of 512
NT = L // 128              # 32 L-tiles of 128
NP = NT // 2               # 16 L-tile pairs

WS = 16.0                  # host weight pre-scale
ESH = -1.5                 # exp shift (softmax-invariant)
QS = 32.0                  # qs = 32*qsm  (fp8 range)
MTS = 0.25                 # mt8 = 0.25 * (256*M) = 64*M
ODS = 1.0 / 2048.0         # output descale: 64*32


def _build_nc():
    import concourse.mybir as mybir
    import concourse.tile as tile
    from concourse import bacc

    F32 = mybir.dt.float32
    F16 = mybir.dt.float16
    F8 = mybir.dt.float8e4
    AF = mybir.ActivationFunctionType
    ALU = mybir.AluOpType
    DR = mybir.MatmulPerfMode.DoubleRow

    nc = bacc.Bacc("TRN2", target_bir_lowering=False, debug=False)

    xin8 = nc.dram_tensor("xin8", [S_PER_CORE * C, L], F8, kind="ExternalInput")
    xin16 = nc.dram_tensor("xin16", [S_PER_CORE * C, L], F16,
                           kind="ExternalInput")
    wk8_d = nc.dram_tensor("wk8", [128, 4, KC], F8, kind="ExternalInput")
    wq8_d = nc.dram_tensor("wq8", [128, 4, KC], F8, kind="ExternalInput")
    wv8_d = nc.dram_tensor("wv8", [128, 4, VC], F8, kind="ExternalInput")
    we8_d = nc.dram_tensor("we8", [128, 4, C], F8, kind="ExternalInput")
    bq_d = nc.dram_tensor("bqv", [128, 2], F32, kind="ExternalInput")
    esh_d = nc.dram_tensor("eshv", [128, 1], F32, kind="ExternalInput")
    bs_d = nc.dram_tensor("bsum8", [128, 128], F8, kind="ExternalInput")
    id_d = nc.dram_tensor("ident", [128, 128], F32, kind="ExternalInput")
    idr_d = nc.dram_tensor("idr16", [128, 128], F16, kind="ExternalInput")
    ones_d = nc.dram_tensor("ones8", [128, 2, 16], F8, kind="ExternalInput")
    xT8_d = nc.dram_tensor("xT8", [S_PER_CORE * L, C], F8, kind="ExternalInput")
    id16_d = nc.dram_tensor("ident16", [128, 128], F16, kind="ExternalInput")
    onr_d = nc.dram_tensor("onesr", [1, 128], F32, kind="ExternalInput")
    out_d = nc.dram_tensor("out", [S_PER_CORE * C, L], F16,
                           kind="ExternalOutput")

    with tile.TileContext(nc) as tc:
        with tc.tile_pool(name="wpool", bufs=1) as wp, \
             tc.tile_pool(name="work", bufs=1) as sp, \
             tc.tile_pool(name="ppool", bufs=1, space="PSUM") as pp:

            # ---- resident weights/constants ----
            wk8 = wp.tile([128, 4, KC], F8, name="wk8_s")
            wq8 = wp.tile([128, 4, KC], F8, name="wq8_s")
            wv8 = wp.tile([128, 4, VC], F8, name="wv8_s")
            we8 = wp.tile([128, 4, C], F8, name="we8_s")
            bsum8 = wp.tile([128, 128], F8, name="bs_s")
            ident = wp.tile([128, 128], F32, name="id_s")
            idr16 = wp.tile([128, 128], F16, name="idr_s")
            ones8 = wp.tile([128, 2, 16], F8, name="ones_s")
            bq2 = wp.tile([128, 2], F32, name="bq_s")
            esh = wp.tile([128, 1], F32, name="esh_s")
            ident16 = wp.tile([128, 128], F16, name="id16_s")
            onesr = wp.tile([1, 128], F32, name="onr_s")

            # first x tile goes out before the weights so DMA queues overlap
            x4_first = sp.tile([128, 4, 512], F8, name="x4_0_0", tag="x4",
                               bufs=4)
            for c in range(4):
                nc.gpsimd.dma_start(
                    out=x4_first[:, c, :],
                    in_=xin8[c * 128:(c + 1) * 128, 0:512])
                nc.sync.dma_start(out=wk8[:, c, :], in_=wk8_d[:, c, :])
                nc.sync.dma_start(out=wv8[:, c, :], in_=wv8_d[:, c, :])
            for dst, src in ((wq8, wq8_d), (we8, we8_d), (bsum8, bs_d),
                             (ident, id_d), (idr16, idr_d), (ones8, ones_d),
                             (bq2, bq_d), (esh, esh_d), (ident16, id16_d),
                             (onesr, onr_d)):
                nc.sync.dma_start(out=dst, in_=src[...])

            # dummy exp to force the one-time ACT_TABLE_LOAD into the
            # DMA lead-in window; memset source so it depends on no DMA
            # (a DMA-dependent warm op inherits the batched DMA semaphore
            # and stalls the whole in-order ACT queue behind the prefetches)
            warm = sp.tile([128, 2], F32, name="actwarm", tag="warm", bufs=1)
            nc.vector.memset(warm[:, :], 0.0)
            nc.scalar.activation(warm[:, 1:2], warm[:, 0:1], AF.Exp,
                                 bias=warm[:, 0:1])

            st = {}   # per-sample state: ctx_ps, skt_ps, mt8

            def load_xT(s, pr):
                xT2 = sp.tile([128, 2, C], F8, name=f"xt{s}_{pr}",
                              tag="xts", bufs=4)
                nc.gpsimd.dma_start(
                    out=xT2,
                    in_=xT8_d[s * L + pr * 256:s * L + (pr + 1) * 256, :]
                    .rearrange("(two p) c -> p two c", p=128))
                st[s][("xt", pr)] = xT2

            bt_pend = []

            def stage1_pair(s, x4, pr, ctx_ps, skt_ps):
                # kt+exp per tile; DoubleRow B^T/skt accumulation is deferred
                # one pair (software pipeline) so exp latency hides under the
                # next pair's kt matmuls
                ekT2 = sp.tile([128, 2, KC], F8, name=f"ek{s}_{pr}",
                               tag="ek", bufs=4)
                xT2 = st[s][("xt", pr)]
                for i in (0, 1):
                    k = (2 * pr + i) % 4          # tile within group
                    ksl = slice(k * 128, (k + 1) * 128)
                    kt_ps = pp.tile([128, KC], F32, name=f"kt{s}_{pr}_{i}",
                                    tag="kt", bufs=2)
                    for cp in (0, 1):
                        cs = slice(2 * cp, 2 * cp + 2)
                        nc.tensor.matmul(kt_ps[:, :], x4[:, cs, ksl],
                                         wk8[:, cs, :], start=(cp == 0),
                                         stop=(cp == 1), perf_mode=DR)
                    nc.scalar.activation(ekT2[:, i, :], kt_ps[:, :], AF.Exp,
                                         bias=esh[:, 0:1], scale=1.0 / WS)
                def bt_accum():
                    for j in (0, 1):
                        jsl = slice(j * 128, (j + 1) * 128)
                        nc.tensor.matmul(ctx_ps[j][:, :], ekT2[:, :, jsl],
                                         xT2[:, :, :],
                                         start=(pr == 0), stop=(pr == NP - 1),
                                         perf_mode=DR)
                    nc.tensor.matmul(skt_ps[:, :], ones8[:, :, 0:1],
                                     ekT2[:, :, :],
                                     start=(pr == 0), stop=(pr == NP - 1),
                                     perf_mode=DR)
                bt_pend.append(bt_accum)
                if len(bt_pend) > 1:
                    bt_pend.pop(0)()
                if pr == NP - 1:
                    while bt_pend:
                        bt_pend.pop(0)()

            def stage1_group(s, g, pairs=(0, 1)):
                row0 = s * C
                if g == 0 and pairs[0] == 0:
                    st[s] = dict(
                        ctx_ps=[pp.tile([128, C], F32, name=f"ctx{s}_{j}",
                                        tag="ctx", bufs=2) for j in range(2)],
                        skt_ps=pp.tile([1, KC], F32, name=f"skt{s}",
                                       tag="skt", bufs=2))
                def load_x4(gg):
                    if s == 0 and gg == 0:
                        st[s][("x4", gg)] = x4_first
                        return
                    t = sp.tile([128, 4, 512], F8, name=f"x4_{s}_{gg}",
                                tag="x4", bufs=4)
                    nc.gpsimd.dma_start(
                        out=t,
                        in_=xin8[row0:row0 + C, gg * 512:(gg + 1) * 512]
                        .rearrange("(c p) l -> p c l", p=128))
                    st[s][("x4", gg)] = t
                if pairs[0] == 0:
                    if g == 0:
                        load_x4(0)
                        load_xT(s, 0)
                        load_xT(s, 1)
                    if g + 1 < NB:
                        load_x4(g + 1)
                        load_xT(s, 2 * g + 2)
                        load_xT(s, 2 * g + 3)
                x4 = st[s][("x4", g)]
                ctx_ps, skt_ps = st[s]["ctx_ps"], st[s]["skt_ps"]
                for p_ in pairs:
                    stage1_pair(s, x4, 2 * g + p_, ctx_ps, skt_ps)

            def mid(s):
                # B-route: bt_ps[j] holds B^T KC-chunk j = [128 k, 512 c].
                # 1) rk row; 2) B^T -> fp8 -> transpose -> B [c-part, k];
                # 3) ctx = (16Wv) @ B per v-chunk; 4) mask+normalize -> cn;
                # 5) mt = cn-pairs @ we8 (DoubleRow), as before.
                bt_ps, skt_ps = st[s]["ctx_ps"], st[s]["skt_ps"]
                sk_sb = sp.tile([1, KC], F32, name=f"sksb{s}", tag="sksb",
                                bufs=2)
                nc.vector.tensor_copy(sk_sb[:, :], skt_ps[:, :])
                rkrow = sp.tile([1, KC], F32, name=f"rkr{s}", tag="rk",
                                bufs=2)
                nc.vector.reciprocal(rkrow[:, :], sk_sb[:, :])
                rkb_ps = pp.tile([128, KC], F32, name=f"rkb{s}", tag="skt",
                                 bufs=2)
                nc.tensor.matmul(rkb_ps[:, :], onesr[0:1, :], rkrow[0:1, :],
                                 start=True, stop=True)
                rkb = sp.tile([128, KC], F32, name=f"rkbs{s}", tag="rkb",
                              bufs=2)
                nc.vector.tensor_copy(rkb[:, :], rkb_ps[:, :])
                btX = sp.tile([128, 2, C], F16, name=f"btx{s}", tag="btx",
                              bufs=2)
                for j in range(2):
                    nc.scalar.activation(btX[:, j, :], bt_ps[j][:, :],
                                         AF.Copy, scale=0.125)
                # transpose B^T -> B: 8 fp8 128x128 transposes
                bX = sp.tile([128, 4, KC], F8, name=f"bx{s}", tag="bx",
                             bufs=2)
                for cc in range(4):
                    csl = slice(cc * 128, (cc + 1) * 128)
                    tr8 = pp.tile([128, 2, 128], F16, name=f"tr8{s}_{cc}",
                                  tag="kt", bufs=2)
                    for j in range(2):
                        nc.tensor.transpose(tr8[:, j, :], btX[:, j, csl],
                                            ident16[:, :])
                    nc.vector.tensor_copy(bX[:, cc, :],
                                          tr8[:, :, :].rearrange(
                                              "p two q -> p (two q)"))
                # ctx[v, k] per v-chunk vc: contract over c (DR pairs)
                cn = sp.tile([128, 4, KC], F8, name=f"cn{s}", tag="cn",
                             bufs=2)
                nc.vector.memset(cn[:, :, :], 0.0)
                for vc in range(4):
                    vsl = slice(vc * 128, (vc + 1) * 128)
                    cx_ps = pp.tile([128, KC], F32, name=f"cx{s}_{vc}",
                                    tag="vt", bufs=2)
                    for ccp in (0, 1):
                        cs = slice(2 * ccp, 2 * ccp + 2)
                        nc.tensor.matmul(cx_ps[:, :], wv8[:, cs, vsl],
                                         bX[:, cs, :], start=(ccp == 0),
                                         stop=(ccp == 1), perf_mode=DR)
                    # normalize+mask the two head blocks living in chunk vc
                    for hh in range(2):
                        h = 2 * vc + hh
                        pr_ = slice(hh * 64, hh * 64 + 64)
                        kr = slice(h * 32, h * 32 + 32)
                        nc.vector.tensor_mul(cn[pr_, vc, kr],
                                             cx_ps[pr_, kr], rkb[pr_, kr])
                mt8 = sp.tile([128, 2, C], F8, name=f"mt{s}", tag="mt",
                              bufs=2)
                for j in range(2):
                    jsl = slice(j * 128, (j + 1) * 128)
                    mt_ps = pp.tile([128, C], F32, name=f"mtp{s}_{j}",
                                    tag="vt", bufs=2)
                    nc.tensor.matmul(mt_ps[:, :], cn[:, 2 * j:2 * j + 2, jsl],
                                     we8[:, 2 * j:2 * j + 2, :],
                                     start=True, stop=True, perf_mode=DR)
                    nc.scalar.activation(mt8[:, j, :], mt_ps[:, :], AF.Copy,
                                         scale=MTS)
                st[s]["mt8"] = mt8

            pend = []

            def _softmaxA(s, b):
                row0 = s * C
                bsl = slice(b * 512, (b + 1) * 512)
                xb8 = sp.tile([128, 4, 512], F8, name=f"xb8{s}_{b}",
                              tag="xb8", bufs=4)
                nc.gpsimd.dma_start(
                    out=xb8,
                    in_=xin8[row0:row0 + C, bsl]
                    .rearrange("(c p) l -> p c l", p=128))
                xb16 = sp.tile([128, 4, 512], F16, name=f"xb{s}_{b}",
                               tag="xb", bufs=4)
                nc.gpsimd.dma_start(
                    out=xb16,
                    in_=xin16[row0:row0 + C, bsl]
                    .rearrange("(c p) l -> p c l", p=128))
                qs2 = sp.tile([128, 2, 512], F8, name=f"qs{s}_{b}",
                              tag="qs", bufs=6)
                eqs = []
                for j in range(2):
                    jsl = slice(j * 128, (j + 1) * 128)
                    q_ps = pp.tile([128, 512], F32, name=f"q{s}_{b}_{j}",
                                   tag="kt", bufs=2)
                    for cp in (0, 1):
                        cs = slice(2 * cp, 2 * cp + 2)
                        nc.tensor.matmul(q_ps[:, :], wq8[:, cs, jsl],
                                         xb8[:, cs, :], start=(cp == 0),
                                         stop=(cp == 1), perf_mode=DR)
                    eq = sp.tile([128, 512], F8, name=f"eq{s}_{b}_{j}",
                                 tag="eq", bufs=6)
                    nc.scalar.activation(eq[:, :], q_ps[:, :], AF.Exp,
                                         bias=bq2[:, j:j + 1],
                                         scale=1.0 / WS)
                    eqs.append(eq)
                for j in range(2):
                    sq_ps = pp.tile([128, 512], F32, name=f"sq{s}_{b}_{j}",
                                    tag="skt", bufs=2)
                    nc.tensor.matmul(sq_ps[:, :], bsum8[:, :], eqs[j][:, :],
                                     start=True, stop=True)
                    rf = sp.tile([128, 512], F32, name=f"rf{s}_{b}_{j}",
                                 tag="rf", bufs=4)
                    nc.vector.reciprocal_approx_fast(rf[:, :], sq_ps[:, :])
                    nc.vector.scalar_tensor_tensor(
                        out=qs2[:, j, :], in0=eqs[j][:, :], scalar=QS,
                        in1=rf[:, :], op0=ALU.mult, op1=ALU.mult)
                return xb16, qs2

            def _outputA(s, b, xb16, qs2):
                row0 = s * C
                mt8 = st[s]["mt8"]
                bsl = slice(b * 512, (b + 1) * 512)
                for c in range(4):
                    o_ps = pp.tile([128, 512], F32, name=f"o{s}_{b}_{c}",
                                   tag="vt", bufs=2)
                    csl = slice(c * 128, (c + 1) * 128)
                    if c < 1:
                        # residual on DVE
                        nc.tensor.matmul(o_ps[:, :], mt8[:, :, csl],
                                         qs2[:, :, :], start=True, stop=True,
                                         perf_mode=DR)
                        oc = sp.tile([128, 512], F16, name=f"oc{s}_{b}_{c}",
                                     tag="oc", bufs=6)
                        nc.vector.scalar_tensor_tensor(
                            out=oc[:, :], in0=o_ps[:, :], scalar=ODS,
                            in1=xb16[:, c, :], op0=ALU.mult, op1=ALU.add)
                    else:
                        # residual folded into PSUM via 2048*I fp16 matmul,
                        # descale via ACT copy
                        nc.tensor.matmul(o_ps[:, :], mt8[:, :, csl],
                                         qs2[:, :, :], start=True, stop=False,
                                         perf_mode=DR)
                        nc.tensor.matmul(o_ps[:, :], idr16[:, :],
                                         xb16[:, c, :],
                                         start=False, stop=True)
                        oc = sp.tile([128, 512], F16, name=f"oc{s}_{b}_{c}",
                                     tag="oc", bufs=6)
                        nc.scalar.activation(oc[:, :], o_ps[:, :], AF.Copy,
                                             scale=ODS)
                    nc.sync.dma_start(
                        out=out_d[row0 + c * 128:row0 + (c + 1) * 128, bsl],
                        in_=oc[:, :])

            def phaseA_bank(s, b):
                pend.append((s, b) + _softmaxA(s, b))
                if len(pend) > 2:
                    _outputA(*pend.pop(0))

            def phaseA_flush():
                while pend:
                    _outputA(*pend.pop(0))

            # schedule: s0 stage1; mid(0); s1 stage1 interleaved with six
            # s0 phase-A banks (PE-rich stage1 complements engine-balanced
            # phase-A); mid(1); tail = s1 banks + s0 banks 6,7
            for g in range(NB):
                stage1_group(0, g)
            mid(0)
            for i in range(NB):
                stage1_group(1, i, pairs=(0,))
                if i >= 2:
                    pend.append((0, i - 2) + _softmaxA(0, i - 2))
                stage1_group(1, i, pairs=(1,))
                if len(pend) > 2:
                    _outputA(*pend.pop(0))
            mid(1)
            for i in range(NB):
                phaseA_bank(1, i)
                if i in (1, 3):
                    phaseA_bank(0, 6 + i // 2)
            phaseA_flush()
    nc.compile()
    return nc


def _host_prep(Wk, bk, Wq, bq, Wv, bv, We, be):
    import ml_dtypes
    f = np.float32
    F8 = ml_dtypes.float8_e4m3

    def chunk8(w):                  # (O, Cin) -> (128, Cin//128, O) fp8, x16
        wt = np.ascontiguousarray(w.T.astype(np.float64) * WS)
        wt = np.clip(wt, -240.0, 240.0)
        nch = wt.shape[0] // 128
        return np.ascontiguousarray(
            wt.reshape(nch, 128, w.shape[0]).transpose(1, 0, 2)).astype(F8)

    wk8 = chunk8(Wk)
    wq8 = chunk8(Wq)
    wv8 = chunk8(Wv)
    we8 = chunk8(We)
    bq2 = np.ascontiguousarray(
        bq.astype(f).reshape(2, 128).T) + np.float32(ESH)
    wb = (We.astype(np.float64) @ bv.astype(np.float64)
          + be.astype(np.float64))
    bsum = np.zeros((128, 128), f)
    for p in range(128):
        bsum[p, (p // 32) * 32:(p // 32) * 32 + 32] = 1.0
    ident = np.eye(128, dtype=f)
    idr16 = (np.eye(128) * 2048.0).astype(np.float16)
    ones8 = np.ones((128, 2, 16), dtype=F8)
    eshv = np.full((128, 1), ESH, dtype=f)
    onesr = np.full((1, 128), 8.0, dtype=f)
    return dict(wk8=wk8, wq8=wq8, wv8=wv8, we8=we8, bqv=bq2, eshv=eshv,
                bsum8=bsum.astype(F8), ident=ident, ident16=ident.astype(np.float16),
                idr16=idr16, ones8=ones8, onesr=onesr), wb


def _make_in_maps(x, Wk, bk, Wq, bq, Wv, bv, We, be):
    import ml_dtypes
    F8 = ml_dtypes.float8_e4m3
    shared, wb = _host_prep(Wk, bk, Wq, bq, Wv, bv, We, be)
    xf = np.ascontiguousarray(x.astype(np.float64).reshape(N, C, L))
    x8 = np.clip(xf, -240.0, 240.0).astype(F8)
    x16 = (xf + wb[None, :, None]).astype(np.float16)
    in_maps = []
    for i in range(N_CORES):
        m = dict(shared)
        sl = slice(i * S_PER_CORE, (i + 1) * S_PER_CORE)
        m["xin8"] = np.ascontiguousarray(
            x8[sl].reshape(S_PER_CORE * C, L))
        m["xin16"] = np.ascontiguousarray(
            x16[sl].reshape(S_PER_CORE * C, L))
        m["xT8"] = np.ascontiguousarray(
            x8[sl].transpose(0, 2, 1).reshape(S_PER_CORE * L, C))
        in_maps.append(m)
    return in_maps


def kernel(x, Wk, bk, Wq, bq, Wv, bv, We, be):
    from concourse.bass_utils import run_bass_kernel_spmd

    assert x.shape == (N, C, Hdim, Wdim), x.shape
    if "nc" not in _CACHE:
        _CACHE["nc"] = _build_nc()
    nc = _CACHE["nc"]

    in_maps = _make_in_maps(x, Wk, bk, Wq, bq, Wv, bv, We, be)
    res = run_bass_kernel_spmd(nc, in_maps, core_ids=list(range(N_CORES)))
    out = np.concatenate(
        [np.asarray(res.results[i]["out"], dtype=np.float32)
         .reshape(S_PER_CORE, C, Hdim, Wdim)
         for i in range(N_CORES)], axis=0)
    return out.astype(np.float32)
